# revision 50
# baseline (speedup 1.0000x reference)
"""MoE FeedForward (top-2 of 4 experts) — expert-parallel Trainium2 kernel.

Strategy (matches the sharding hint): the tiny gating matmul + top-k routing
run on host as part of input sharding; tokens are dispatched by gate index to
expert-owning cores (expert e -> cores 2e, 2e+1, each taking half of that
expert's tokens, padded to a common capacity C). Each core computes
    y^T = (relu(W1^T @ x^T + b1) -> W2^T @ mid + b2) * gate
entirely in transposed [feature, token] layout (no on-device transposes),
with bf16 matmuls accumulating in fp32 PSUM. The host combine scatter-adds
the two gate-weighted expert contributions per token.

Fast path (used whenever b1 == b2 == 0, which holds for this problem): the
positive gate is folded into x on host (relu(g*z) == g*relu(z) for g > 0),
removing the on-device gate multiply and its [128, C] gate-broadcast load.

Model dims (hardcoded per problem spec): N=8192 tokens, D=512, H=2048,
E=4 experts, top-k=2, 8 NeuronCores.
"""

import numpy as np
import ml_dtypes
from contextlib import ExitStack

D = 512
H = 2048
E = 4
TOP_K = 2
N_CORES = 8
ND = D // 128   # 4 d-tiles
NH = H // 128   # 16 h-tiles
HB = 4          # w1 column-block = 512 cols = 4 h-tiles

_NC_CACHE = {}


def _chunk_plan(C: int, chunk: int = 512):
    # k near-equal chunks (width <= 512 = one fp32 PSUM bank); equal widths
    # keep every matmul far from the small-N issue floor
    k = -(-C // chunk)
    base, rem = divmod(C, k)
    chunks = []
    off = 0
    for i in range(k):
        s = base + (1 if i < rem else 0)
        chunks.append((off, s))
        off += s
    return chunks


def _build_moe_nc(C: int, fold_gate: bool):
    """Per-core SPMD program: [D,C] bf16 tokens -> [D,C] f32 expert output."""
    import concourse.mybir as mybir
    from concourse import bacc, tile

    dt = mybir.dt
    AF = mybir.ActivationFunctionType

    assert C % 8 == 0
    chunks = _chunk_plan(C)

    nc = bacc.Bacc(None, target_bir_lowering=False)
    # host pre-arranges every input partition-major so each DMA below is a
    # flat, fully contiguous [128, K] copy (max SDMA bandwidth, min
    # descriptor count):
    #   xt: chunk-major blocks, inside a block di-major: [128, ND*S per chunk]
    #   w1: hb-major blocks of [128, ND*512]
    #   w2: wb-major blocks of [128, 8*512]
    S0 = chunks[0][1]
    # head = w1's first two h-tiles + chunk0's tokens, fused into ONE DMA so
    # the first GEMM group's operands arrive with a single fixed-cost latency
    head = nc.dram_tensor("head", [128, ND * 256 + ND * S0], dt.bfloat16,
                          kind="ExternalInput")
    w1b = nc.dram_tensor("w1b", [128, ND * 256], dt.bfloat16,
                         kind="ExternalInput")
    if C > S0:
        xt = nc.dram_tensor("xt", [128, ND * (C - S0)], dt.bfloat16,
                            kind="ExternalInput")
    w1 = nc.dram_tensor("w1", [128, (HB - 1) * ND * 512], dt.bfloat16,
                        kind="ExternalInput")
    w2 = nc.dram_tensor("w2", [128, 2 * 8 * 512], dt.bfloat16,
                        kind="ExternalInput")
    if not fold_gate:
        b1r = nc.dram_tensor("b1r", [128, NH], dt.float32, kind="ExternalInput")
        b2r = nc.dram_tensor("b2r", [128, ND], dt.float32, kind="ExternalInput")
        gr = nc.dram_tensor("gr", [128, C], dt.float32, kind="ExternalInput")
    # output, chunk-major like xt: per chunk a [128, ND*S] contiguous block
    # (one DMA per chunk); host unpacks back to [N, D] token rows
    yt = nc.dram_tensor("yt", [128, ND * C], dt.float32, kind="ExternalOutput")

    # flat offset of each chunk's block inside xt (chunk0 lives in head) / yt
    xt_off = {}
    yt_off = {}
    acc = 0
    yacc = 0
    for i, (c0, S) in enumerate(chunks):
        if i > 0:
            xt_off[c0] = acc
            acc += ND * S
        yt_off[c0] = yacc
        yacc += ND * S

    with tile.TileContext(nc) as tc, ExitStack() as ctx:
        wpool = ctx.enter_context(tc.tile_pool(name="weights", bufs=1))
        xpool = ctx.enter_context(tc.tile_pool(name="x", bufs=1))
        midp = ctx.enter_context(tc.tile_pool(name="mid", bufs=34))
        p1 = ctx.enter_context(tc.tile_pool(name="p1", bufs=4, space="PSUM"))
        p2 = ctx.enter_context(tc.tile_pool(name="p2", bufs=3, space="PSUM"))
        pw = ctx.enter_context(tc.tile_pool(name="pw", bufs=1, space="PSUM"))
        ypool = ctx.enter_context(tc.tile_pool(name="y", bufs=6))

        # PE warm-up: dummy matmuls spanning the engine preamble + first
        # input DMAs so the HAM clock gate is at full rate when real matmuls
        # begin. Output bank is never read.
        warm_sb = wpool.tile([128, 128], dt.bfloat16, tag="warm", name="warm_sb")
        nc.vector.memset(warm_sb[:], 0.0)
        warm_ps = pw.tile([128, 64], dt.float32, tag="warm_ps", name="warm_ps")
        for _ in range(56):
            nc.tensor.matmul(warm_ps[:], warm_sb[:], warm_sb[:, :64],
                             start=True, stop=True, skip_group_check=True)

        # Input loads: few large DMAs (each InstDMACopy is split across all
        # 16 SDMA engines), emitted in need-time order on the sync HWDGE
        # ring. Tiny bias loads go on the scalar HWDGE ring.
        xt_sb = {}

        def load_xt(c0, S):
            t = xpool.tile([128, ND * S], dt.bfloat16,
                           tag=f"xt_{c0}", name=f"xt_{c0}")
            o = xt_off[c0]
            nc.sync.dma_start(t[:], xt[:, o:o + ND * S])
            xt_sb[c0] = t

        # head: [w1 h-tiles 0-1 | xt chunk 0] — one DMA, one fixed-cost latency
        head_sb = wpool.tile([128, ND * 256 + ND * S0], dt.bfloat16,
                             tag="head", name="head_sb")
        nc.sync.dma_start(head_sb[:], head[:])
        w1b_sb = wpool.tile([128, ND * 256], dt.bfloat16, tag="w1b", name="w1b_sb")
        nc.sync.dma_start(w1b_sb[:], w1b[:])
        xt_sb[chunks[0][0]] = None  # resolved via head in gemm1

        w1_sb = {}

        def load_w1(hb):
            t = wpool.tile([128, ND * 512], dt.bfloat16,
                           tag=f"w1_{hb}", name=f"w1_{hb}")
            o = (hb - 1) * ND * 512
            nc.sync.dma_start(t[:], w1[:, o:o + ND * 512])
            w1_sb[hb] = t

        for hb in range(1, HB):
            load_w1(hb)

        def w1_lhsT(ht, di):
            if ht < 2:
                return head_sb[:, di * 256 + ht * 128:di * 256 + (ht + 1) * 128]
            if ht < 4:
                o = di * 256 + (ht - 2) * 128
                return w1b_sb[:, o:o + 128]
            o = di * 512 + (ht % HB) * 128
            return w1_sb[ht // HB][:, o:o + 128]
        if len(chunks) > 1:
            load_xt(*chunks[1])
        if len(chunks) > 2:
            load_xt(*chunks[2])
        if not fold_gate:
            b1_sb = wpool.tile([128, NH], dt.float32, tag="b1", name="b1_sb")
            nc.scalar.dma_start(b1_sb[:], b1r[:])
            b2_sb = wpool.tile([128, ND], dt.float32, tag="b2", name="b2_sb")
            nc.scalar.dma_start(b2_sb[:], b2r[:])
        w2_sb = []
        for wb in range(2):
            t = wpool.tile([128, 8 * 512], dt.bfloat16,
                           tag=f"w2_{wb}", name=f"w2_{wb}")
            o = wb * 8 * 512
            nc.sync.dma_start(t[:], w2[:, o:o + 8 * 512])
            w2_sb.append(t)
        for (c0, S) in chunks[3:]:
            load_xt(c0, S)
        if not fold_gate:
            gr_sb = wpool.tile([128, C], dt.float32, tag="gr", name="gr_sb")
            nc.sync.dma_start(gr_sb[:], gr[:])

        def gemm1(c0, S):
            # mid^T[h, c] = relu(sum_d w1[d,h] * x^T[d,c] (+ b1[h]))
            mids = []
            for ht in range(NH):
                ps = p1.tile([128, S], dt.float32, tag="ps1", name=f"ps1_{c0}_{ht}")
                for di in range(ND):
                    if xt_sb[c0] is None:   # chunk0 rides in the head tile
                        rhs = head_sb[:, ND * 256 + di * S:ND * 256 + (di + 1) * S]
                    else:
                        rhs = xt_sb[c0][:, di * S:(di + 1) * S]
                    nc.tensor.matmul(
                        ps[:],
                        w1_lhsT(ht, di),
                        rhs,
                        start=(di == 0),
                        stop=(di == ND - 1),
                    )
                m = midp.tile([128, S], dt.bfloat16, tag="mid", name=f"mid_{c0}_{ht}")
                if fold_gate:
                    nc.scalar.activation(m[:], ps[:], AF.Relu)
                else:
                    nc.scalar.activation(m[:], ps[:], AF.Relu,
                                         bias=b1_sb[:, ht:ht + 1])
                mids.append(m)
            return mids

        def gemm2(c0, S, mids):
            # y^T[d, c] = (sum_h w2[h,d] * mid^T[h,c] (+ b2[d])) (* g[c])
            o = yt_off[c0]
            for di in range(ND):
                ps2 = p2.tile([128, S], dt.float32, tag="ps2", name=f"ps2_{c0}_{di}")
                for ht in range(NH):
                    wo = (ht % 8) * 512 + di * 128
                    nc.tensor.matmul(
                        ps2[:],
                        w2_sb[ht // 8][:, wo:wo + 128],
                        mids[ht][:],
                        start=(ht == 0),
                        stop=(ht == NH - 1),
                    )
                yt_t = ypool.tile([128, S], dt.float32, tag="y", name=f"y_{c0}_{di}")
                if fold_gate:
                    nc.scalar.activation(yt_t[:], ps2[:], AF.Copy)
                else:
                    nc.scalar.activation(yt_t[:], ps2[:], AF.Identity,
                                         bias=b2_sb[:, di:di + 1])
                    nc.vector.tensor_mul(yt_t[:], yt_t[:], gr_sb[:, c0:c0 + S])
                nc.sync.dma_start(yt[:, o + di * S:o + (di + 1) * S], yt_t[:])

        # software-pipeline by one chunk: GEMM1 of chunk i+1 is emitted before
        # GEMM2 of chunk i, giving the PE dense work while w2 streams in
        prev = None
        for (c0, S) in chunks:
            mids = gemm1(c0, S)
            if prev is not None:
                gemm2(*prev)
            prev = (c0, S, mids)
        gemm2(*prev)

    nc.finalize()
    return nc


def _route(h, w_gate):
    """Top-2 gating, matching jax.lax.top_k (ties -> lower index) + softmax."""
    logits = h @ w_gate                                      # [N, E] f32
    order = np.argsort(-logits, axis=1, kind="stable")
    top_idx = order[:, :TOP_K]                               # [N, 2]
    top_lg = np.take_along_axis(logits, top_idx, axis=1)
    mx = top_lg.max(axis=1, keepdims=True)
    ex = np.exp(top_lg - mx)
    gates2 = (ex / ex.sum(axis=1, keepdims=True)).astype(np.float32)
    return top_idx, gates2


def _run(inputs, trace=False):
    from concourse.bass_utils import run_bass_kernel_spmd

    bf16 = ml_dtypes.bfloat16
    h = np.asarray(inputs["h"], dtype=np.float32)
    w_gate = np.asarray(inputs["w_gate"], dtype=np.float32)
    w1 = np.asarray(inputs["w1"], dtype=np.float32)
    b1 = np.asarray(inputs["b1"], dtype=np.float32)
    w2 = np.asarray(inputs["w2"], dtype=np.float32)
    b2 = np.asarray(inputs["b2"], dtype=np.float32)
    N = h.shape[0]

    fold_gate = not (b1.any() or b2.any())
    top_idx, gates2 = _route(h, w_gate)

    # dispatch: expert e -> cores 2e (first half) and 2e+1 (second half)
    core_toks, core_gates, core_expert = [], [], []
    for e in range(E):
        sel = top_idx == e                                   # [N, 2] bool
        toks = np.nonzero(sel.any(axis=1))[0]
        g = gates2[toks, sel[toks].argmax(axis=1)]
        half = (len(toks) + 1) // 2
        for lo, hi in ((0, half), (half, len(toks))):
            core_toks.append(toks[lo:hi])
            core_gates.append(g[lo:hi])
            core_expert.append(e)

    maxlen = max(len(t) for t in core_toks)
    C = max(128, -(-maxlen // 8) * 8)

    key = (C, fold_gate)
    if key not in _NC_CACHE:
        _NC_CACHE[key] = _build_moe_nc(C, fold_gate)
    nc = _NC_CACHE[key]

    chunks = _chunk_plan(C)

    # partition-major packers matching the kernel's flat DMA layouts
    def pack_w1(e, h0, h1):
        return (w1[e].astype(bf16).reshape(ND, 128, H)[:, :, h0:h1]
                .transpose(1, 0, 2).reshape(128, ND * (h1 - h0)))

    w1_head = {}
    w1b_packed = {}
    w1_packed = {}
    w2_packed = {}
    for e in set(core_expert):
        w1_head[e] = pack_w1(e, 0, 256)
        w1b_packed[e] = np.ascontiguousarray(pack_w1(e, 256, 512))
        w1_packed[e] = np.concatenate(
            [pack_w1(e, hb * 512, (hb + 1) * 512) for hb in range(1, HB)], axis=1)
        w2_packed[e] = np.ascontiguousarray(
            w2[e].astype(bf16).reshape(2, 8, 128, 512)
            .transpose(2, 0, 1, 3).reshape(128, 2 * 8 * 512))

    in_maps = []
    for c in range(N_CORES):
        e = core_expert[c]
        toks = core_toks[c]
        n = len(toks)
        xtT = np.zeros((D, C), dtype=bf16)
        if fold_gate:
            xtT[:, :n] = (h[toks] * core_gates[c][:, None]).T.astype(bf16)
        else:
            xtT[:, :n] = h[toks].T.astype(bf16)
        r = xtT.reshape(ND, 128, C)

        def xt_block(c0, S):
            return r[:, :, c0:c0 + S].transpose(1, 0, 2).reshape(128, ND * S)

        S0 = chunks[0][1]
        head_arr = np.empty((128, ND * 256 + ND * S0), dtype=bf16)
        head_arr[:, :ND * 256] = w1_head[e]
        head_arr[:, ND * 256:] = xt_block(*chunks[0])
        im = {
            "head": head_arr,
            "w1b": w1b_packed[e],
            "w1": w1_packed[e],
            "w2": w2_packed[e],
        }
        if C > S0:
            xt_arr = np.empty((128, ND * (C - S0)), dtype=bf16)
            o = 0
            for (c0, S) in chunks[1:]:
                xt_arr[:, o:o + ND * S] = xt_block(c0, S)
                o += ND * S
            im["xt"] = xt_arr
        if not fold_gate:
            grow = np.zeros(C, dtype=np.float32)
            grow[:n] = core_gates[c]
            im["b1r"] = np.ascontiguousarray(b1[e].reshape(NH, 128).T)
            im["b2r"] = np.ascontiguousarray(b2[e].reshape(ND, 128).T)
            im["gr"] = np.ascontiguousarray(np.broadcast_to(grow, (128, C)))
        in_maps.append(im)

    res = run_bass_kernel_spmd(nc, in_maps, core_ids=list(range(N_CORES)),
                               trace=trace)

    out = np.zeros((N, D), dtype=np.float32)
    for c in range(N_CORES):
        toks = core_toks[c]
        if not len(toks):
            continue
        # unpack chunk-major [128, ND*C] back to y^T [D, C]
        raw = res.results[c]["yt"]
        ytT = np.empty((D, C), dtype=np.float32)
        o = 0
        for (c0, S) in chunks:
            ytT[:, c0:c0 + S] = (
                raw[:, o:o + ND * S].reshape(128, ND, S)
                .transpose(1, 0, 2).reshape(D, S))
            o += ND * S
        out[toks] += ytT[:, :len(toks)].T
    return out, res


def kernel(**inputs) -> np.ndarray:
    out, _ = _run(inputs, trace=False)
    return out


# revision 51
# speedup vs baseline: 1.0231x; 1.0231x over previous
"""MoE FeedForward (top-2 of 4 experts) — expert-parallel Trainium2 kernel.

Strategy (matches the sharding hint): the tiny gating matmul + top-k routing
run on host as part of input sharding; tokens are dispatched by gate index to
expert-owning cores (expert e -> cores 2e, 2e+1, each taking half of that
expert's tokens, padded to a common capacity C). Each core computes
    y^T = (relu(W1^T @ x^T + b1) -> W2^T @ mid + b2) * gate
entirely in transposed [feature, token] layout (no on-device transposes),
with bf16 matmuls accumulating in fp32 PSUM. The host combine scatter-adds
the two gate-weighted expert contributions per token.

Fast path (used whenever b1 == b2 == 0, which holds for this problem): the
positive gate is folded into x on host (relu(g*z) == g*relu(z) for g > 0),
removing the on-device gate multiply and its [128, C] gate-broadcast load.

Model dims (hardcoded per problem spec): N=8192 tokens, D=512, H=2048,
E=4 experts, top-k=2, 8 NeuronCores.
"""

import numpy as np
import ml_dtypes
from contextlib import ExitStack

D = 512
H = 2048
E = 4
TOP_K = 2
N_CORES = 8
ND = D // 128   # 4 d-tiles
NH = H // 128   # 16 h-tiles
HB = 4          # w1 column-block = 512 cols = 4 h-tiles

_NC_CACHE = {}


def _chunk_plan(C: int, chunk: int = 512):
    # k near-equal chunks (width <= 512 = one fp32 PSUM bank); equal widths
    # keep every matmul far from the small-N issue floor
    k = -(-C // chunk)
    base, rem = divmod(C, k)
    chunks = []
    off = 0
    for i in range(k):
        s = base + (1 if i < rem else 0)
        chunks.append((off, s))
        off += s
    return chunks


def _build_moe_nc(C: int, fold_gate: bool):
    """Per-core SPMD program: [D,C] bf16 tokens -> [D,C] f32 expert output."""
    import concourse.mybir as mybir
    from concourse import bacc, tile

    dt = mybir.dt
    AF = mybir.ActivationFunctionType

    assert C % 2 == 0
    chunks = _chunk_plan(C)

    nc = bacc.Bacc(None, target_bir_lowering=False)
    # host pre-arranges every input partition-major so each DMA below is a
    # flat, fully contiguous [128, K] copy (max SDMA bandwidth, min
    # descriptor count):
    #   xt: chunk-major blocks, inside a block di-major: [128, ND*S per chunk]
    #   w1: hb-major blocks of [128, ND*512]
    #   w2: wb-major blocks of [128, 8*512]
    S0 = chunks[0][1]
    # head = w1's first two h-tiles + chunk0's tokens, fused into ONE DMA so
    # the first GEMM group's operands arrive with a single fixed-cost latency
    head = nc.dram_tensor("head", [128, ND * 256 + ND * S0], dt.bfloat16,
                          kind="ExternalInput")
    w1b = nc.dram_tensor("w1b", [128, ND * 256], dt.bfloat16,
                         kind="ExternalInput")
    if C > S0:
        xt = nc.dram_tensor("xt", [128, ND * (C - S0)], dt.bfloat16,
                            kind="ExternalInput")
    w1 = nc.dram_tensor("w1", [128, (HB - 1) * ND * 512], dt.bfloat16,
                        kind="ExternalInput")
    w2 = nc.dram_tensor("w2", [128, 2 * 8 * 512], dt.bfloat16,
                        kind="ExternalInput")
    if not fold_gate:
        b1r = nc.dram_tensor("b1r", [128, NH], dt.float32, kind="ExternalInput")
        b2r = nc.dram_tensor("b2r", [128, ND], dt.float32, kind="ExternalInput")
        gr = nc.dram_tensor("gr", [128, C], dt.float32, kind="ExternalInput")
    # output, chunk-major like xt: per chunk a [128, ND*S] contiguous block
    # (one DMA per chunk); host unpacks back to [N, D] token rows
    yt = nc.dram_tensor("yt", [128, ND * C], dt.float32, kind="ExternalOutput")

    # flat offset of each chunk's block inside xt (chunk0 lives in head) / yt
    xt_off = {}
    yt_off = {}
    acc = 0
    yacc = 0
    for i, (c0, S) in enumerate(chunks):
        if i > 0:
            xt_off[c0] = acc
            acc += ND * S
        yt_off[c0] = yacc
        yacc += ND * S

    with tile.TileContext(nc) as tc, ExitStack() as ctx:
        wpool = ctx.enter_context(tc.tile_pool(name="weights", bufs=1))
        xpool = ctx.enter_context(tc.tile_pool(name="x", bufs=1))
        midp = ctx.enter_context(tc.tile_pool(name="mid", bufs=34))
        p1 = ctx.enter_context(tc.tile_pool(name="p1", bufs=4, space="PSUM"))
        p2 = ctx.enter_context(tc.tile_pool(name="p2", bufs=3, space="PSUM"))
        pw = ctx.enter_context(tc.tile_pool(name="pw", bufs=1, space="PSUM"))
        ypool = ctx.enter_context(tc.tile_pool(name="y", bufs=6))

        # PE warm-up: dummy matmuls spanning the engine preamble + first
        # input DMAs so the HAM clock gate is at full rate when real matmuls
        # begin. Output bank is never read.
        warm_sb = wpool.tile([128, 128], dt.bfloat16, tag="warm", name="warm_sb")
        nc.vector.memset(warm_sb[:], 0.0)
        warm_ps = pw.tile([128, 64], dt.float32, tag="warm_ps", name="warm_ps")
        for _ in range(56):
            nc.tensor.matmul(warm_ps[:], warm_sb[:], warm_sb[:, :64],
                             start=True, stop=True, skip_group_check=True)

        # Input loads: few large DMAs (each InstDMACopy is split across all
        # 16 SDMA engines), emitted in need-time order on the sync HWDGE
        # ring. Tiny bias loads go on the scalar HWDGE ring.
        xt_sb = {}

        def load_xt(c0, S):
            t = xpool.tile([128, ND * S], dt.bfloat16,
                           tag=f"xt_{c0}", name=f"xt_{c0}")
            o = xt_off[c0]
            nc.sync.dma_start(t[:], xt[:, o:o + ND * S])
            xt_sb[c0] = t

        # head: [w1 h-tiles 0-1 | xt chunk 0] — one DMA, one fixed-cost latency
        head_sb = wpool.tile([128, ND * 256 + ND * S0], dt.bfloat16,
                             tag="head", name="head_sb")
        nc.sync.dma_start(head_sb[:], head[:])
        w1b_sb = wpool.tile([128, ND * 256], dt.bfloat16, tag="w1b", name="w1b_sb")
        nc.sync.dma_start(w1b_sb[:], w1b[:])
        xt_sb[chunks[0][0]] = None  # resolved via head in gemm1

        w1_sb = {}

        def load_w1(hb):
            t = wpool.tile([128, ND * 512], dt.bfloat16,
                           tag=f"w1_{hb}", name=f"w1_{hb}")
            o = (hb - 1) * ND * 512
            nc.sync.dma_start(t[:], w1[:, o:o + ND * 512])
            w1_sb[hb] = t

        for hb in range(1, HB):
            load_w1(hb)

        def w1_lhsT(ht, di):
            if ht < 2:
                return head_sb[:, di * 256 + ht * 128:di * 256 + (ht + 1) * 128]
            if ht < 4:
                o = di * 256 + (ht - 2) * 128
                return w1b_sb[:, o:o + 128]
            o = di * 512 + (ht % HB) * 128
            return w1_sb[ht // HB][:, o:o + 128]
        if len(chunks) > 1:
            load_xt(*chunks[1])
        if len(chunks) > 2:
            load_xt(*chunks[2])
        if not fold_gate:
            b1_sb = wpool.tile([128, NH], dt.float32, tag="b1", name="b1_sb")
            nc.scalar.dma_start(b1_sb[:], b1r[:])
            b2_sb = wpool.tile([128, ND], dt.float32, tag="b2", name="b2_sb")
            nc.scalar.dma_start(b2_sb[:], b2r[:])
        w2_sb = []
        for wb in range(2):
            t = wpool.tile([128, 8 * 512], dt.bfloat16,
                           tag=f"w2_{wb}", name=f"w2_{wb}")
            o = wb * 8 * 512
            nc.sync.dma_start(t[:], w2[:, o:o + 8 * 512])
            w2_sb.append(t)
        for (c0, S) in chunks[3:]:
            load_xt(c0, S)
        if not fold_gate:
            gr_sb = wpool.tile([128, C], dt.float32, tag="gr", name="gr_sb")
            nc.sync.dma_start(gr_sb[:], gr[:])

        def gemm1(c0, S):
            # mid^T[h, c] = relu(sum_d w1[d,h] * x^T[d,c] (+ b1[h]))
            mids = []
            for ht in range(NH):
                ps = p1.tile([128, S], dt.float32, tag="ps1", name=f"ps1_{c0}_{ht}")
                for di in range(ND):
                    if xt_sb[c0] is None:   # chunk0 rides in the head tile
                        rhs = head_sb[:, ND * 256 + di * S:ND * 256 + (di + 1) * S]
                    else:
                        rhs = xt_sb[c0][:, di * S:(di + 1) * S]
                    nc.tensor.matmul(
                        ps[:],
                        w1_lhsT(ht, di),
                        rhs,
                        start=(di == 0),
                        stop=(di == ND - 1),
                    )
                m = midp.tile([128, S], dt.bfloat16, tag="mid", name=f"mid_{c0}_{ht}")
                if fold_gate:
                    nc.scalar.activation(m[:], ps[:], AF.Relu)
                else:
                    nc.scalar.activation(m[:], ps[:], AF.Relu,
                                         bias=b1_sb[:, ht:ht + 1])
                mids.append(m)
            return mids

        def gemm2(c0, S, mids):
            # y^T[d, c] = (sum_h w2[h,d] * mid^T[h,c] (+ b2[d])) (* g[c])
            o = yt_off[c0]
            for di in range(ND):
                ps2 = p2.tile([128, S], dt.float32, tag="ps2", name=f"ps2_{c0}_{di}")
                for ht in range(NH):
                    wo = (ht % 8) * 512 + di * 128
                    nc.tensor.matmul(
                        ps2[:],
                        w2_sb[ht // 8][:, wo:wo + 128],
                        mids[ht][:],
                        start=(ht == 0),
                        stop=(ht == NH - 1),
                    )
                yt_t = ypool.tile([128, S], dt.float32, tag="y", name=f"y_{c0}_{di}")
                if fold_gate:
                    nc.scalar.activation(yt_t[:], ps2[:], AF.Copy)
                else:
                    nc.scalar.activation(yt_t[:], ps2[:], AF.Identity,
                                         bias=b2_sb[:, di:di + 1])
                    nc.vector.tensor_mul(yt_t[:], yt_t[:], gr_sb[:, c0:c0 + S])
                nc.sync.dma_start(yt[:, o + di * S:o + (di + 1) * S], yt_t[:])

        # software-pipeline by one chunk: GEMM1 of chunk i+1 is emitted before
        # GEMM2 of chunk i, giving the PE dense work while w2 streams in
        prev = None
        for (c0, S) in chunks:
            mids = gemm1(c0, S)
            if prev is not None:
                gemm2(*prev)
            prev = (c0, S, mids)
        gemm2(*prev)

    nc.finalize()
    return nc


def _route(h, w_gate):
    """Top-2 gating, matching jax.lax.top_k (ties -> lower index) + softmax."""
    logits = h @ w_gate                                      # [N, E] f32
    order = np.argsort(-logits, axis=1, kind="stable")
    top_idx = order[:, :TOP_K]                               # [N, 2]
    top_lg = np.take_along_axis(logits, top_idx, axis=1)
    mx = top_lg.max(axis=1, keepdims=True)
    ex = np.exp(top_lg - mx)
    gates2 = (ex / ex.sum(axis=1, keepdims=True)).astype(np.float32)
    return top_idx, gates2


def _run(inputs, trace=False):
    from concourse.bass_utils import run_bass_kernel_spmd

    bf16 = ml_dtypes.bfloat16
    h = np.asarray(inputs["h"], dtype=np.float32)
    w_gate = np.asarray(inputs["w_gate"], dtype=np.float32)
    w1 = np.asarray(inputs["w1"], dtype=np.float32)
    b1 = np.asarray(inputs["b1"], dtype=np.float32)
    w2 = np.asarray(inputs["w2"], dtype=np.float32)
    b2 = np.asarray(inputs["b2"], dtype=np.float32)
    N = h.shape[0]

    fold_gate = not (b1.any() or b2.any())
    top_idx, gates2 = _route(h, w_gate)

    # dispatch: expert e -> cores 2e (first half) and 2e+1 (second half)
    core_toks, core_gates, core_expert = [], [], []
    for e in range(E):
        sel = top_idx == e                                   # [N, 2] bool
        toks = np.nonzero(sel.any(axis=1))[0]
        g = gates2[toks, sel[toks].argmax(axis=1)]
        half = (len(toks) + 1) // 2
        for lo, hi in ((0, half), (half, len(toks))):
            core_toks.append(toks[lo:hi])
            core_gates.append(g[lo:hi])
            core_expert.append(e)

    maxlen = max(len(t) for t in core_toks)
    C = max(128, -(-maxlen // 2) * 2)

    key = (C, fold_gate)
    if key not in _NC_CACHE:
        _NC_CACHE[key] = _build_moe_nc(C, fold_gate)
    nc = _NC_CACHE[key]

    chunks = _chunk_plan(C)

    # partition-major packers matching the kernel's flat DMA layouts
    def pack_w1(e, h0, h1):
        return (w1[e].astype(bf16).reshape(ND, 128, H)[:, :, h0:h1]
                .transpose(1, 0, 2).reshape(128, ND * (h1 - h0)))

    w1_head = {}
    w1b_packed = {}
    w1_packed = {}
    w2_packed = {}
    for e in set(core_expert):
        w1_head[e] = pack_w1(e, 0, 256)
        w1b_packed[e] = np.ascontiguousarray(pack_w1(e, 256, 512))
        w1_packed[e] = np.concatenate(
            [pack_w1(e, hb * 512, (hb + 1) * 512) for hb in range(1, HB)], axis=1)
        w2_packed[e] = np.ascontiguousarray(
            w2[e].astype(bf16).reshape(2, 8, 128, 512)
            .transpose(2, 0, 1, 3).reshape(128, 2 * 8 * 512))

    in_maps = []
    for c in range(N_CORES):
        e = core_expert[c]
        toks = core_toks[c]
        n = len(toks)
        xtT = np.zeros((D, C), dtype=bf16)
        if fold_gate:
            xtT[:, :n] = (h[toks] * core_gates[c][:, None]).T.astype(bf16)
        else:
            xtT[:, :n] = h[toks].T.astype(bf16)
        r = xtT.reshape(ND, 128, C)

        def xt_block(c0, S):
            return r[:, :, c0:c0 + S].transpose(1, 0, 2).reshape(128, ND * S)

        S0 = chunks[0][1]
        head_arr = np.empty((128, ND * 256 + ND * S0), dtype=bf16)
        head_arr[:, :ND * 256] = w1_head[e]
        head_arr[:, ND * 256:] = xt_block(*chunks[0])
        im = {
            "head": head_arr,
            "w1b": w1b_packed[e],
            "w1": w1_packed[e],
            "w2": w2_packed[e],
        }
        if C > S0:
            xt_arr = np.empty((128, ND * (C - S0)), dtype=bf16)
            o = 0
            for (c0, S) in chunks[1:]:
                xt_arr[:, o:o + ND * S] = xt_block(c0, S)
                o += ND * S
            im["xt"] = xt_arr
        if not fold_gate:
            grow = np.zeros(C, dtype=np.float32)
            grow[:n] = core_gates[c]
            im["b1r"] = np.ascontiguousarray(b1[e].reshape(NH, 128).T)
            im["b2r"] = np.ascontiguousarray(b2[e].reshape(ND, 128).T)
            im["gr"] = np.ascontiguousarray(np.broadcast_to(grow, (128, C)))
        in_maps.append(im)

    res = run_bass_kernel_spmd(nc, in_maps, core_ids=list(range(N_CORES)),
                               trace=trace)

    out = np.zeros((N, D), dtype=np.float32)
    for c in range(N_CORES):
        toks = core_toks[c]
        if not len(toks):
            continue
        # unpack chunk-major [128, ND*C] back to y^T [D, C]
        raw = res.results[c]["yt"]
        ytT = np.empty((D, C), dtype=np.float32)
        o = 0
        for (c0, S) in chunks:
            ytT[:, c0:c0 + S] = (
                raw[:, o:o + ND * S].reshape(128, ND, S)
                .transpose(1, 0, 2).reshape(D, S))
            o += ND * S
        out[toks] += ytT[:, :len(toks)].T
    return out, res


def kernel(**inputs) -> np.ndarray:
    out, _ = _run(inputs, trace=False)
    return out
